# revision 3
# baseline (speedup 1.0000x reference)
"""ADRNN (2x 2-layer LSTM + linears) Trainium2 Bass kernel, 8-core SPMD.

One uniform SPMD program; core 0 carries the r-LSTM pair (r0, r1 + Wr
linear), core 1 carries the t-LSTM pair (t0, t1 + Wt linear), selected
purely by per-core weight/input data. Cores 2-7 run the same program on
zero data. Per core the two LSTM layers are interleaved with a one-step
lag so their serial cell chains overlap across engines.

Layout: batch (128) on partitions, gates/hidden on the free dim. Matmuls
stream bf16 weights as the moving operand (full rate), with the per-step
h state DMA-transposed (bf16) to serve as the stationary lhsT. The
r_out -> t_input edge moves once per 32-step group via a pair-wise
AllGather; the t core's whole timeline is shifted by one group host-side
(its warmup group sees all-zero inputs including the bias carrier rows,
so its state stays exactly zero until the real sequence starts).
"""

import numpy as np
import ml_dtypes

import concourse.tile as tile
from concourse import bacc, mybir
from concourse.bass_utils import run_bass_kernel_spmd

F32 = mybir.dt.float32
BF16 = mybir.dt.bfloat16
AF = mybir.ActivationFunctionType

H = 512
R = 47
TR = 2
B = 128
T = 512
S = 32            # steps per group
NCORES = 8
KT = H // 128     # 4 k-tiles for H
G4 = 4 * H

bf = ml_dtypes.bfloat16


# ---------------------------------------------------------------- host prep

def _reorder_gates(w, b):
    """torch gate order (i,f,g,o) -> (g,i,f,o). w:[4H, K], b:[4H]."""
    i, f, g, o = (w[k * H:(k + 1) * H] for k in range(4))
    bi, bff, bg, bo = (b[k * H:(k + 1) * H] for k in range(4))
    return np.concatenate([g, i, f, o], 0), np.concatenate([bg, bi, bff, bo], 0)


def _pack_pair(Wih0, b0, WihE, Whh0, Wih1, b1, Whh1, Wproj, bproj):
    """Pack one LSTM pair's weights into the uniform per-core tensor dict."""
    d = {}
    nx = Wih0.shape[1]
    w_ihX = np.zeros((128, G4), np.float32)
    w_ihX[:nx] = Wih0.T
    w_ihX[49] = b0                      # rides the xT "ones" row
    if WihE is not None:
        w_ihX[64:64 + WihE.shape[1]] = WihE.T
    d["w_ihX"] = w_ihX
    for name, w in (("w_hh0", Whh0), ("w_ih1", Wih1), ("w_hh1", Whh1)):
        wt = w.T.astype(np.float32)     # [H, 4H]
        d[name] = np.concatenate([wt[k * 128:(k + 1) * 128] for k in range(KT)],
                                 axis=1)  # [128, KT*4H]
    d["b1row"] = b1.reshape(1, G4).astype(np.float32)
    wp = np.zeros((H, 64), np.float32)
    wp[:, :Wproj.shape[0]] = Wproj.T
    d["w_proj"] = np.concatenate([wp[k * 128:(k + 1) * 128] for k in range(KT)],
                                 axis=1)  # [128, KT*64]
    bp = np.zeros((1, 64), np.float32)
    bp[0, :bproj.shape[0]] = bproj
    d["b_projrow"] = bp
    return d


def _pack_x(x_cat, shift_groups, G, Tl):
    """x_cat: [Tl, B, nx] time-major inputs (f32) -> xe [G+1, 64, S*128]."""
    nx = x_cat.shape[2]
    xe = np.zeros((G + 1, 64, S * 128), np.float32)
    for gi in range(G + 1):
        for s in range(S):
            t_real = gi * S + s - shift_groups * S
            if 0 <= t_real < Tl:
                blk = xe[gi, :, s * 128:(s + 1) * 128]
                blk[:nx] = x_cat[t_real].T
                blk[49] = 1.0
    return xe


def _build_core_inputs(inputs):
    x_r, x_t = np.asarray(inputs["x_r"]), np.asarray(inputs["x_t"])
    Tl = x_r.shape[1]
    G = Tl // S
    xc = np.concatenate([x_r, x_t], axis=2).transpose(1, 0, 2).astype(np.float32)

    rW0, rb0 = _reorder_gates(np.asarray(inputs["r_Wih0"]),
                              np.asarray(inputs["r_b0"]))
    rWh0, _ = _reorder_gates(np.asarray(inputs["r_Whh0"]), np.zeros(G4))
    rW1, rb1 = _reorder_gates(np.asarray(inputs["r_Wih1"]),
                              np.asarray(inputs["r_b1"]))
    rWh1, _ = _reorder_gates(np.asarray(inputs["r_Whh1"]), np.zeros(G4))
    r_w = _pack_pair(rW0, rb0, None, rWh0, rW1, rb1, rWh1,
                     np.asarray(inputs["Wr"]), np.asarray(inputs["br"]))
    r_w["xe"] = _pack_x(xc, 0, G, Tl)

    tW0, tb0 = _reorder_gates(np.asarray(inputs["t_Wih0"]),
                              np.asarray(inputs["t_b0"]))
    tWh0, _ = _reorder_gates(np.asarray(inputs["t_Whh0"]), np.zeros(G4))
    tW1, tb1 = _reorder_gates(np.asarray(inputs["t_Wih1"]),
                              np.asarray(inputs["t_b1"]))
    tWh1, _ = _reorder_gates(np.asarray(inputs["t_Whh1"]), np.zeros(G4))
    t_w = _pack_pair(tW0[:, :49], tb0, tW0[:, 49:96], tWh0, tW1, tb1, tWh1,
                     np.asarray(inputs["Wt"]), np.asarray(inputs["bt"]))
    t_w["xe"] = _pack_x(xc, 1, G, Tl)

    zero_w = {k: np.zeros_like(v) for k, v in r_w.items()}

    def to_map(d):
        return {k: np.ascontiguousarray(v.astype(bf)) for k, v in d.items()}

    maps = [to_map(r_w), to_map(t_w)] + [to_map(zero_w)] * (NCORES - 2)
    return maps, G, Tl


# ---------------------------------------------------------------- builder

def build_nc(G):
    nc = bacc.Bacc("TRN2", target_bir_lowering=False, debug=False,
                   num_devices=NCORES)

    xe = nc.dram_tensor("xe", [G + 1, 64, S * 128], BF16, kind="ExternalInput")
    w_ihX_d = nc.dram_tensor("w_ihX", [128, G4], BF16, kind="ExternalInput")
    w_hh0_d = nc.dram_tensor("w_hh0", [128, KT * G4], BF16, kind="ExternalInput")
    w_ih1_d = nc.dram_tensor("w_ih1", [128, KT * G4], BF16, kind="ExternalInput")
    w_hh1_d = nc.dram_tensor("w_hh1", [128, KT * G4], BF16, kind="ExternalInput")
    b1_d = nc.dram_tensor("b1row", [1, G4], BF16, kind="ExternalInput")
    w_pj_d = nc.dram_tensor("w_proj", [128, KT * 64], BF16, kind="ExternalInput")
    b_pj_d = nc.dram_tensor("b_projrow", [1, 64], BF16, kind="ExternalInput")

    out = nc.dram_tensor("out", [G + 1, S, B, 64], F32, kind="ExternalOutput")

    RG = [[0, 1], [2, 3], [4, 5], [6, 7]]
    SP = S // 2  # packed projT tiles per group (2 steps per [128,128] tile)

    with tile.TileContext(nc) as tc:
        with (
            tc.tile_pool(name="wpool", bufs=1) as wpool,
            tc.tile_pool(name="state", bufs=2) as state,
            tc.tile_pool(name="cell", bufs=2) as cell,
            tc.tile_pool(name="stage", bufs=2) as stage,
            tc.tile_pool(name="psum", bufs=1, space="PSUM") as psum,
            tc.tile_pool(name="dram", bufs=2, space="DRAM") as dpool,
        ):
            # ---- load weights into SBUF once
            w_ihX = wpool.tile([128, G4], BF16, tag="wihX")
            nc.sync.dma_start(w_ihX[:], w_ihX_d[:])
            w_hh0 = wpool.tile([128, KT * G4], BF16, tag="whh0")
            nc.sync.dma_start(w_hh0[:], w_hh0_d[:])
            w_ih1 = wpool.tile([128, KT * G4], BF16, tag="wih1")
            nc.sync.dma_start(w_ih1[:], w_ih1_d[:])
            w_hh1 = wpool.tile([128, KT * G4], BF16, tag="whh1")
            nc.sync.dma_start(w_hh1[:], w_hh1_d[:])
            b1 = wpool.tile([1, G4], BF16, tag="b1")
            nc.sync.dma_start(b1[:], b1_d[:])
            w_pj = wpool.tile([128, KT * 64], BF16, tag="wproj")
            nc.sync.dma_start(w_pj[:], w_pj_d[:])
            b_pj = wpool.tile([1, 64], BF16, tag="bproj")
            nc.sync.dma_start(b_pj[:], b_pj_d[:])
            ones = wpool.tile([1, 128], BF16, tag="ones")
            nc.vector.memset(ones[:], 1.0)

            # ---- persistent state (zero-init)
            h0T = state.tile([128, H], BF16, tag="h0T")
            h1T = state.tile([128, H], BF16, tag="h1T")
            c0 = state.tile([128, H], F32, tag="c0")
            c1 = state.tile([128, H], F32, tag="c1")
            for st in (h0T, h1T, c0, c1):
                nc.vector.memset(st[:], 0.0)

            # initial (zero) edge send buffer for group -1
            zed = stage.tile([128, SP * 128], BF16, tag="projT")
            nc.vector.memset(zed[:], 0.0)
            send_prev = dpool.tile([128, SP * 128], BF16, tag="send")
            nc.sync.dma_start(send_prev[:], zed[:])

            def lstm_step(is_l1, inT, rcol, whh, hT_prev, c_st, h0T_for_l1):
                """Gates + cell for one layer-step. Returns (c_new, hT_new)."""
                lid = "1" if is_l1 else "0"
                pg = psum.tile([128, G4], F32, tag="g" + lid)
                for n in range(4):
                    ns = slice(n * 512, (n + 1) * 512)
                    if not is_l1:
                        nc.tensor.matmul(pg[:, ns],
                                         inT[:, rcol * 128:(rcol + 1) * 128],
                                         w_ihX[:, ns],
                                         start=True, stop=False,
                                         skip_group_check=True)
                    else:
                        nc.tensor.matmul(pg[:, ns], ones[:], b1[:, ns],
                                         start=True, stop=False,
                                         skip_group_check=True)
                        for k in range(KT):
                            nc.tensor.matmul(
                                pg[:, ns],
                                h0T_for_l1[:, k * 128:(k + 1) * 128],
                                w_ih1[:, k * G4 + n * 512:k * G4 + (n + 1) * 512],
                                start=False, stop=False,
                                skip_group_check=True)
                    for k in range(KT):
                        nc.tensor.matmul(
                            pg[:, ns],
                            hT_prev[:, k * 128:(k + 1) * 128],
                            whh[:, k * G4 + n * 512:k * G4 + (n + 1) * 512],
                            start=False, stop=(k == KT - 1),
                            skip_group_check=True)
                tg = cell.tile([128, 512], F32, tag="tg" + lid)
                nc.scalar.activation(tg[:], pg[:, 0:512], AF.Tanh)
                sf = cell.tile([128, 1536], F32, tag="sf" + lid)
                nc.scalar.activation(sf[:], pg[:, 512:2048], AF.Sigmoid)
                t2 = cell.tile([128, 512], F32, tag="t2_" + lid)
                nc.vector.tensor_mul(t2[:], sf[:, 0:512], tg[:])
                t1 = cell.tile([128, 512], F32, tag="t1_" + lid)
                nc.vector.tensor_mul(t1[:], sf[:, 512:1024], c_st[:])
                c_new = state.tile([128, H], F32, tag="c" + lid)
                nc.vector.tensor_add(c_new[:], t1[:], t2[:])
                tch = cell.tile([128, 512], F32, tag="tc" + lid)
                nc.scalar.activation(tch[:], c_new[:], AF.Tanh)
                h_new = cell.tile([128, 512], BF16, tag="h" + lid)
                nc.vector.tensor_mul(h_new[:], sf[:, 1024:1536], tch[:])
                hT_new = state.tile([128, H], BF16, tag="h" + lid + "T")
                for k in range(KT):
                    nc.sync.dma_start_transpose(
                        hT_new[:, k * 128:(k + 1) * 128],
                        h_new[:, k * 128:(k + 1) * 128])
                return c_new, hT_new

            def proj_step(h1T_new, pj_f32, pj_bf, s):
                pp = psum.tile([128, 64], F32, tag="g1")
                nc.tensor.matmul(pp[:], ones[:], b_pj[:],
                                 start=True, stop=False, skip_group_check=True)
                for k in range(KT):
                    nc.tensor.matmul(pp[:], h1T_new[:, k * 128:(k + 1) * 128],
                                     w_pj[:, k * 64:(k + 1) * 64],
                                     start=False, stop=(k == KT - 1),
                                     skip_group_check=True)
                nc.scalar.copy(pj_f32[:, s * 64:(s + 1) * 64], pp[:])
                nc.vector.tensor_copy(pj_bf[:, s * 64:(s + 1) * 64], pp[:])

            for gi in range(G + 1):
                # edge AllGather: ships my group gi-1 projT; partner's arrives
                recv = dpool.tile([2 * 128, SP * 128], BF16, tag="recv")
                nc.gpsimd.collective_compute(
                    "AllGather", mybir.AluOpType.bypass,
                    ins=[send_prev.opt()], outs=[recv.opt()],
                    replica_groups=RG)

                # per-group input lhsT buffer: x rows 0:64, edge rows 64:128
                inT = stage.tile([128, S * 128], BF16, tag="inT")
                nc.sync.dma_start(inT[0:64, :], xe[gi])
                # partner's packed projT: tile s2 rows 0:64 = step 2*s2,
                # rows 64:128 = step 2*s2+1
                prt = recv[0:128, :].rearrange("p (s2 c) -> p s2 c", c=128)
                dst = inT[64:128, :].rearrange("p (s2 two c) -> p s2 two c",
                                               two=2, c=128)
                nc.sync.dma_start(dst[:, :, 0, :], prt[0:64])
                nc.sync.dma_start(dst[:, :, 1, :], prt[64:128])

                pj_f32 = stage.tile([128, S * 64], F32, tag="pjf")
                pj_bf = stage.tile([128, S * 64], BF16, tag="pjb")
                pj_bfT = stage.tile([128, SP * 128], BF16, tag="projT")

                h0T_l1 = h0T  # h0T(step n-1), input to L1 at round r
                for r in range(S):
                    c0, h0T = lstm_step(False, inT, r, w_hh0, h0T, c0, None)
                    if not (gi == 0 and r == 0) and r >= 1:
                        s1 = r - 1
                        c1, h1T = lstm_step(True, None, 0, w_hh1, h1T, c1,
                                            h0T_l1)
                        proj_step(h1T, pj_f32, pj_bf, s1)
                        if s1 % 2 == 1:
                            nc.sync.dma_start_transpose(
                                pj_bfT[:, (s1 // 2) * 128:(s1 // 2 + 1) * 128],
                                pj_bf[:, (s1 - 1) * 64:(s1 + 1) * 64])
                    h0T_l1 = h0T
                # flush round: L1 for the group's last step
                c1, h1T = lstm_step(True, None, 0, w_hh1, h1T, c1, h0T_l1)
                proj_step(h1T, pj_f32, pj_bf, S - 1)
                nc.sync.dma_start_transpose(
                    pj_bfT[:, (SP - 1) * 128:SP * 128],
                    pj_bf[:, (S - 2) * 64:S * 64])

                # outputs + edge send staging
                nc.sync.dma_start(
                    out[gi].rearrange("s p c -> p s c"),
                    pj_f32[:].rearrange("p (s c) -> p s c", c=64))
                send_prev = dpool.tile([128, SP * 128], BF16, tag="send")
                nc.sync.dma_start(send_prev[:], pj_bfT[:])

    nc.compile()
    return nc


# ---------------------------------------------------------------- entry

_CACHE = {}


def _get_nc(G):
    if G not in _CACHE:
        _CACHE[G] = build_nc(G)
    return _CACHE[G]


def kernel(**inputs):
    maps, G, Tl = _build_core_inputs(inputs)
    nc = _get_nc(G)
    res = run_bass_kernel_spmd(nc, maps, core_ids=list(range(NCORES)))
    out0 = res.results[0]["out"]   # r core
    out1 = res.results[1]["out"]   # t core (shifted by 1 group)
    r_seq = out0[:G].reshape(G * S, B, 64)[:, :, :R].transpose(1, 0, 2)
    t_seq = out1[1:G + 1].reshape(G * S, B, 64)[:, :, :TR].transpose(1, 0, 2)
    return (np.ascontiguousarray(r_seq.astype(np.float32)),
            np.ascontiguousarray(t_seq.astype(np.float32)))


# revision 18
# speedup vs baseline: 10.4756x; 10.4756x over previous
"""ADRNN (2x 2-layer LSTM + linears) Trainium2 Bass kernel, 8-core SPMD.

One uniform SPMD program; core 0 carries the r-LSTM pair (r0, r1 + Wr
linear), core 1 carries the t-LSTM pair (t0, t1 + Wt linear), selected
purely by per-core weight/input data. Cores 2-7 run the same program on
zero data. Per core the two LSTM layers are interleaved with a one-step
lag so their serial cell chains overlap across engines.

Layout: batch (128) on partitions, gates/hidden on the free dim. Matmuls
stream bf16 weights as the moving operand (full rate), with the per-step
h state DMA-transposed (bf16) to serve as the stationary lhsT. The
r_out -> t_input edge moves once per 32-step group via a pair-wise
AllGather; the t core's whole timeline is shifted by one group host-side
(its warmup group sees all-zero inputs including the bias carrier rows,
so its state stays exactly zero until the real sequence starts).
"""

import numpy as np
import ml_dtypes

import concourse.tile as tile
from concourse import bacc, mybir
from concourse.bass_utils import run_bass_kernel_spmd

F32 = mybir.dt.float32
BF16 = mybir.dt.bfloat16
AF = mybir.ActivationFunctionType

H = 512
R = 47
TR = 2
B = 128
T = 512
S = 32            # steps per group
LAG = 2           # groups of slack for the edge AllGather (async overlap)
NCORES = 8
KT = H // 128     # 4 k-tiles for H
G4 = 4 * H

bf = ml_dtypes.bfloat16


# ---------------------------------------------------------------- host prep

def _reorder_gates(w, b):
    """torch gate order (i,f,g,o) -> (g,i,f,o). w:[4H, K], b:[4H]."""
    i, f, g, o = (w[k * H:(k + 1) * H] for k in range(4))
    bi, bff, bg, bo = (b[k * H:(k + 1) * H] for k in range(4))
    return np.concatenate([g, i, f, o], 0), np.concatenate([bg, bi, bff, bo], 0)


def _pack_pair(Wih0, b0, WihE, Whh0, Wih1, b1, Whh1, Wproj, bproj):
    """Pack one LSTM pair's weights into the uniform per-core tensor dict."""
    d = {}
    nx = Wih0.shape[1]
    w_ihX = np.zeros((128, G4), np.float32)
    w_ihX[:nx] = Wih0.T
    w_ihX[49] = b0                      # rides the xT "ones" row
    if WihE is not None:
        w_ihX[64:64 + WihE.shape[1]] = WihE.T
    d["w_ihX"] = w_ihX
    for name, w in (("w_hh0", Whh0), ("w_ih1", Wih1), ("w_hh1", Whh1)):
        wt = w.T.astype(np.float32)     # [H, 4H]
        d[name] = np.concatenate([wt[k * 128:(k + 1) * 128] for k in range(KT)],
                                 axis=1)  # [128, KT*4H]
    d["b1row"] = b1.reshape(1, G4).astype(np.float32)
    wp = np.zeros((H, 64), np.float32)
    wp[:, :Wproj.shape[0]] = Wproj.T
    d["w_proj"] = np.concatenate([wp[k * 128:(k + 1) * 128] for k in range(KT)],
                                 axis=1)  # [128, KT*64]
    bp = np.zeros((1, 64), np.float32)
    bp[0, :bproj.shape[0]] = bproj
    d["b_projrow"] = bp
    return d


def _pack_x(x_cat, shift_groups, G, Tl):
    """x_cat: [Tl, B, nx] time-major inputs (f32) -> xe [G+LAG, 64, S*128]."""
    nx = x_cat.shape[2]
    xe = np.zeros((G + LAG, 64, S * 128), np.float32)
    for gi in range(G + LAG):
        for s in range(S):
            t_real = gi * S + s - shift_groups * S
            if 0 <= t_real < Tl:
                blk = xe[gi, :, s * 128:(s + 1) * 128]
                blk[:nx] = x_cat[t_real].T
                blk[49] = 1.0
    return xe


def _build_core_inputs(inputs):
    x_r, x_t = np.asarray(inputs["x_r"]), np.asarray(inputs["x_t"])
    Tl = x_r.shape[1]
    G = Tl // S
    xc = np.concatenate([x_r, x_t], axis=2).transpose(1, 0, 2).astype(np.float32)

    rW0, rb0 = _reorder_gates(np.asarray(inputs["r_Wih0"]),
                              np.asarray(inputs["r_b0"]))
    rWh0, _ = _reorder_gates(np.asarray(inputs["r_Whh0"]), np.zeros(G4))
    rW1, rb1 = _reorder_gates(np.asarray(inputs["r_Wih1"]),
                              np.asarray(inputs["r_b1"]))
    rWh1, _ = _reorder_gates(np.asarray(inputs["r_Whh1"]), np.zeros(G4))
    r_w = _pack_pair(rW0, rb0, None, rWh0, rW1, rb1, rWh1,
                     np.asarray(inputs["Wr"]), np.asarray(inputs["br"]))
    r_w["xe"] = _pack_x(xc, 0, G, Tl)
    # noqa: t core below is shifted by LAG groups

    tW0, tb0 = _reorder_gates(np.asarray(inputs["t_Wih0"]),
                              np.asarray(inputs["t_b0"]))
    tWh0, _ = _reorder_gates(np.asarray(inputs["t_Whh0"]), np.zeros(G4))
    tW1, tb1 = _reorder_gates(np.asarray(inputs["t_Wih1"]),
                              np.asarray(inputs["t_b1"]))
    tWh1, _ = _reorder_gates(np.asarray(inputs["t_Whh1"]), np.zeros(G4))
    t_w = _pack_pair(tW0[:, :49], tb0, tW0[:, 49:96], tWh0, tW1, tb1, tWh1,
                     np.asarray(inputs["Wt"]), np.asarray(inputs["bt"]))
    t_w["xe"] = _pack_x(xc, LAG, G, Tl)

    zero_w = {k: np.zeros_like(v) for k, v in r_w.items()}

    def to_map(d):
        return {k: np.ascontiguousarray(v.astype(bf)) for k, v in d.items()}

    maps = [to_map(r_w), to_map(t_w)] + [to_map(zero_w)] * (NCORES - 2)
    return maps, G, Tl


# ---------------------------------------------------------------- builder

def build_nc(G, reps=1, variant=""):
    """variant: comma-separated flags for timing bisection:
    nocc (skip AllGather), nocell (skip ACT/DVE cell; copy gates->h),
    nol1 (skip layer-1 + proj), plaindma (copy instead of transpose)."""
    v = set(variant.split(",")) if variant else set()
    nc = bacc.Bacc("TRN2", target_bir_lowering=False, debug=False,
                   num_devices=NCORES)

    xe = nc.dram_tensor("xe", [G + LAG, 64, S * 128], BF16, kind="ExternalInput")
    w_ihX_d = nc.dram_tensor("w_ihX", [128, G4], BF16, kind="ExternalInput")
    w_hh0_d = nc.dram_tensor("w_hh0", [128, KT * G4], BF16, kind="ExternalInput")
    w_ih1_d = nc.dram_tensor("w_ih1", [128, KT * G4], BF16, kind="ExternalInput")
    w_hh1_d = nc.dram_tensor("w_hh1", [128, KT * G4], BF16, kind="ExternalInput")
    b1_d = nc.dram_tensor("b1row", [1, G4], BF16, kind="ExternalInput")
    w_pj_d = nc.dram_tensor("w_proj", [128, KT * 64], BF16, kind="ExternalInput")
    b_pj_d = nc.dram_tensor("b_projrow", [1, 64], BF16, kind="ExternalInput")

    out = nc.dram_tensor("out", [G + LAG, S, B, 64], F32, kind="ExternalOutput")

    RG = [[0, 1], [2, 3], [4, 5], [6, 7]]
    SP = S // 2  # packed projT tiles per group (2 steps per [128,128] tile)

    with tile.TileContext(nc) as tc:
        with (
            tc.tile_pool(name="wpool", bufs=1) as wpool,
            tc.tile_pool(name="state", bufs=2) as state,
            tc.tile_pool(name="cell", bufs=2) as cell,
            tc.tile_pool(name="stage", bufs=2) as stage,
            tc.tile_pool(name="psum", bufs=1, space="PSUM") as psum,
            tc.tile_pool(name="dram", bufs=2, space="DRAM") as dpool,
        ):
            # ---- load weights into SBUF once
            w_ihX = wpool.tile([128, G4], BF16, tag="wihX")
            nc.sync.dma_start(w_ihX[:], w_ihX_d[:])
            w_hh0 = wpool.tile([128, KT * G4], BF16, tag="whh0")
            nc.sync.dma_start(w_hh0[:], w_hh0_d[:])
            w_ih1 = wpool.tile([128, KT * G4], BF16, tag="wih1")
            nc.sync.dma_start(w_ih1[:], w_ih1_d[:])
            w_hh1 = wpool.tile([128, KT * G4], BF16, tag="whh1")
            nc.sync.dma_start(w_hh1[:], w_hh1_d[:])
            b1 = wpool.tile([1, G4], BF16, tag="b1")
            nc.sync.dma_start(b1[:], b1_d[:])
            w_pj = wpool.tile([128, KT * 64], BF16, tag="wproj")
            nc.sync.dma_start(w_pj[:], w_pj_d[:])
            b_pj = wpool.tile([1, 64], BF16, tag="bproj")
            nc.sync.dma_start(b_pj[:], b_pj_d[:])
            ones = wpool.tile([1, 128], BF16, tag="ones")
            nc.vector.memset(ones[:], 1.0)

            # ---- persistent state (zero-init)
            h0T = state.tile([128, H], BF16, tag="h0T")
            h1T = state.tile([128, H], BF16, tag="h1T")
            c0 = state.tile([128, H], F32, tag="c0")
            c1 = state.tile([128, H], F32, tag="c1")
            for st in (h0T, h1T, c0, c1):
                nc.vector.memset(st[:], 0.0)

            # initial (zero) edge send buffers for warmup groups
            zed = stage.tile([128, SP * 128], BF16, tag="projT")
            nc.vector.memset(zed[:], 0.0)
            send_hist = []
            for _ in range(LAG):
                sz = dpool.tile([128, SP * 128], BF16, tag="send", bufs=LAG + 1)
                nc.sync.dma_start(sz[:], zed[:])
                send_hist.append(sz)

            def lstm_step(is_l1, inT, rcol, whh, hT_prev, c_st, h0T_for_l1):
                """Gates + cell for one layer-step. Returns (c_new, hT_new)."""
                lid = "1" if is_l1 else "0"
                pg = []
                for n in range(4):
                    ns = slice(n * 512, (n + 1) * 512)
                    pb = psum.tile([128, 512], F32, tag=f"g{lid}{n}")
                    pg.append(pb)
                    if not is_l1:
                        nc.tensor.matmul(pb[:],
                                         inT[:, rcol * 128:(rcol + 1) * 128],
                                         w_ihX[:, ns],
                                         start=True, stop=False,
                                         skip_group_check=True)
                    else:
                        nc.tensor.matmul(pb[:], ones[:], b1[:, ns],
                                         start=True, stop=False,
                                         skip_group_check=True)
                        for k in range(KT):
                            nc.tensor.matmul(
                                pb[:],
                                h0T_for_l1[:, k * 128:(k + 1) * 128],
                                w_ih1[:, k * G4 + n * 512:k * G4 + (n + 1) * 512],
                                start=False, stop=False,
                                skip_group_check=True)
                    for k in range(KT):
                        nc.tensor.matmul(
                            pb[:],
                            hT_prev[:, k * 128:(k + 1) * 128],
                            whh[:, k * G4 + n * 512:k * G4 + (n + 1) * 512],
                            start=False, stop=(k == KT - 1),
                            skip_group_check=True)
                if "nocell" in v:
                    c_new = c_st
                    h_new = cell.tile([128, 512], BF16, tag="h" + lid)
                    nc.vector.tensor_copy(h_new[:], pg[0][:])
                else:
                    # gate order in PSUM: (g, i, f, o); split sigmoids so the
                    # c-chain starts as soon as each operand lands
                    tg = cell.tile([128, 512], F32, tag="tg" + lid)
                    nc.scalar.activation(tg[:], pg[0][:], AF.Tanh)
                    si = cell.tile([128, 512], F32, tag="si" + lid)
                    nc.scalar.activation(si[:], pg[1][:], AF.Sigmoid)
                    t2 = cell.tile([128, 512], F32, tag="t2_" + lid)
                    nc.vector.tensor_mul(t2[:], si[:], tg[:])
                    sf = cell.tile([128, 512], F32, tag="sf" + lid)
                    nc.scalar.activation(sf[:], pg[2][:], AF.Sigmoid)
                    t1 = cell.tile([128, 512], F32, tag="t1_" + lid)
                    nc.vector.tensor_mul(t1[:], sf[:], c_st[:])
                    c_new = state.tile([128, H], F32, tag="c" + lid)
                    nc.vector.tensor_add(c_new[:], t1[:], t2[:])
                    so = cell.tile([128, 512], F32, tag="so" + lid)
                    nc.scalar.activation(so[:], pg[3][:], AF.Sigmoid)
                    tch = cell.tile([128, 512], F32, tag="tc" + lid)
                    nc.scalar.activation(tch[:], c_new[:], AF.Tanh)
                    h_new = cell.tile([128, 512], BF16, tag="h" + lid)
                    nc.vector.tensor_mul(h_new[:], so[:], tch[:])
                hT_new = state.tile([128, H], BF16, tag="h" + lid + "T")
                if "dvet" in v:       # timing probe only (wrong data)
                    nc.vector.tensor_copy(hT_new[:], h_new[:])
                else:
                    nkt = 1 if "tr1" in v else KT
                    for k in range(nkt):
                        if "plaindma" in v:
                            nc.sync.dma_start(
                                hT_new[:, k * 128:(k + 1) * 128],
                                h_new[:, k * 128:(k + 1) * 128])
                        else:
                            nc.sync.dma_start_transpose(
                                hT_new[:, k * 128:(k + 1) * 128],
                                h_new[:, k * 128:(k + 1) * 128])
                    if "tr1" in v:
                        nc.vector.tensor_copy(hT_new[:, 128:], h_new[:, 128:])
                return c_new, hT_new

            def proj_step(h1T_new, pj_f32, pj_bf, s):
                pp = psum.tile([128, 64], F32, tag="g10")
                nc.tensor.matmul(pp[:], ones[:], b_pj[:],
                                 start=True, stop=False, skip_group_check=True)
                for k in range(KT):
                    nc.tensor.matmul(pp[:], h1T_new[:, k * 128:(k + 1) * 128],
                                     w_pj[:, k * 64:(k + 1) * 64],
                                     start=False, stop=(k == KT - 1),
                                     skip_group_check=True)
                nc.scalar.copy(pj_f32[:, s * 64:(s + 1) * 64], pp[:])
                nc.vector.tensor_copy(pj_bf[:, s * 64:(s + 1) * 64], pp[:])

            for gi in [g for _ in range(reps) for g in range(G + LAG)]:
                # edge AllGather: ships my group gi-1 projT; partner's arrives
                recv = dpool.tile([2 * 128, SP * 128], BF16, tag="recv")
                if "nocc" not in v:
                    nc.gpsimd.collective_compute(
                        "AllGather", mybir.AluOpType.bypass,
                        ins=[send_hist[-LAG].opt()], outs=[recv.opt()],
                        replica_groups=RG)

                # per-group input lhsT buffer: x rows 0:64, edge rows 64:128
                inT = stage.tile([128, S * 128], BF16, tag="inT")
                nc.sync.dma_start(inT[0:64, :], xe[gi])
                # partner's packed projT: tile s2 rows 0:64 = step 2*s2,
                # rows 64:128 = step 2*s2+1
                prt = recv[0:128, :].rearrange("p (s2 c) -> p s2 c", c=128)
                dst = inT[64:128, :].rearrange("p (s2 two c) -> p s2 two c",
                                               two=2, c=128)
                nc.sync.dma_start(dst[:, :, 0, :], prt[0:64])
                nc.sync.dma_start(dst[:, :, 1, :], prt[64:128])

                pj_f32 = stage.tile([128, S * 64], F32, tag="pjf")
                pj_bf = stage.tile([128, S * 64], BF16, tag="pjb")
                pj_bfT = stage.tile([128, SP * 128], BF16, tag="projT")

                h0T_l1 = h0T  # h0T(step n-1), input to L1 at round r
                for r in range(S):
                    c0, h0T = lstm_step(False, inT, r, w_hh0, h0T, c0, None)
                    if r >= 1 and "nol1" not in v:
                        s1 = r - 1
                        c1, h1T = lstm_step(True, None, 0, w_hh1, h1T, c1,
                                            h0T_l1)
                        proj_step(h1T, pj_f32, pj_bf, s1)
                        if s1 % 2 == 1:
                            nc.sync.dma_start_transpose(
                                pj_bfT[:, (s1 // 2) * 128:(s1 // 2 + 1) * 128],
                                pj_bf[:, (s1 - 1) * 64:(s1 + 1) * 64])
                    h0T_l1 = h0T
                if "nol1" not in v:
                    # flush round: L1 for the group's last step
                    c1, h1T = lstm_step(True, None, 0, w_hh1, h1T, c1, h0T_l1)
                    proj_step(h1T, pj_f32, pj_bf, S - 1)
                    nc.sync.dma_start_transpose(
                        pj_bfT[:, (SP - 1) * 128:SP * 128],
                        pj_bf[:, (S - 2) * 64:S * 64])

                # outputs + edge send staging
                nc.sync.dma_start(
                    out[gi].rearrange("s p c -> p s c"),
                    pj_f32[:].rearrange("p (s c) -> p s c", c=64))
                send_new = dpool.tile([128, SP * 128], BF16, tag="send",
                                      bufs=LAG + 1)
                nc.sync.dma_start(send_new[:], pj_bfT[:])
                send_hist.append(send_new)

    nc.compile()
    return nc


# ---------------------------------------------------------------- entry

_CACHE = {}


def _get_nc(G, reps=1, variant=""):
    key = (G, reps, variant)
    if key not in _CACHE:
        _CACHE[key] = build_nc(G, reps, variant)
    return _CACHE[key]


def kernel(**inputs):
    maps, G, Tl = _build_core_inputs(inputs)
    nc = _get_nc(G)
    res = run_bass_kernel_spmd(nc, maps, core_ids=list(range(NCORES)))
    out0 = res.results[0]["out"]   # r core
    out1 = res.results[1]["out"]   # t core (shifted by 1 group)
    r_seq = out0[:G].reshape(G * S, B, 64)[:, :, :R].transpose(1, 0, 2)
    t_seq = out1[LAG:G + LAG].reshape(G * S, B, 64)[:, :, :TR].transpose(1, 0, 2)
    return (np.ascontiguousarray(r_seq.astype(np.float32)),
            np.ascontiguousarray(t_seq.astype(np.float32)))
